# revision 4
# baseline (speedup 1.0000x reference)
"""Trainium2 Bass kernel: per-channel 8x8 box-sum pooling, stride 4 (NCHW).

Input  x: (8, 32, 512, 512) f32  ->  output (8, 32, 127, 127) f32.

Sharding: data-parallel over the batch dim — image b runs on NeuronCore b
(zero communication).

Per core (32 channel planes of 512 x 512):

  1. Input DMA, 2 planes per dma_start (16 total): SBUF layout [128, e*512]
     with partition p holding input rows 4p..4p+3 — each partition's span is
     8 KiB of *contiguous* DRAM, so descriptors are 8 KiB and each 1-MiB
     plane stripes across all 16 DMA engines (64-KiB chunk round-robin).
  2. Vertical pooling on the tensor engine in f32r: with 4 consecutive rows
     per partition, window i covers exactly partitions i and i+1, for every
     row-chunk e. So all 4 accumulating matmuls share one bidiagonal weight
     matrix M[p, i] = (p==i) | (p==i+1):  V[i, w] = sum_e (M.T @ X_e)[i, w]
     = sum_{dh<8} x[4i+dh, w].  f32r rounds the data operand's mantissa
     (weights are exact 0/1); rel err ~1e-4 vs the 2e-2 gate.
  3. Horizontal pooling on the vector engine, reading PSUM [128, 512]:
     pairwise tree a[u]=V[2u]+V[2u+1]; b[m]=a[2m]+a[2m+1];
     out[i,j]=b[j]+b[j+1] — 511 output elems/partition/plane.
  4. The final add writes into an SBUF staging tile [128, 16*128] (row i of
     plane c at free offset (c%16)*128). Two 1-MiB stores (after plane 15
     and plane 31) write DRAM out[i, c, j] — i-major, so each partition's
     16 planes are 8 KiB contiguous -> 8 KiB descriptors striped across all
     DMA engines, instead of 32 per-plane 65-KiB stores that all land on
     DMA engine 0. The host transposes [128, 32, 128] -> [32, 127, 127].

Row 127 of V (weight column 127 is all-zero) and output column 127 carry
zeros/garbage; the host slices both pads off.
"""

import numpy as np

B, C, H, W = 8, 32, 512, 512
KS, ST = 8, 4
HO = (H - KS) // ST + 1  # 127
WO = (W - KS) // ST + 1  # 127
P = 128
E = H // P  # 4 rows per partition
G = 2  # planes per input DMA
SG = C // 2  # planes per output store group

_CACHE: dict = {}


def _pool_matrix() -> np.ndarray:
    # M[p, i] = 1 iff output row i's 8-row window covers partition p's rows
    # (4p..4p+3), i.e. p == i or p == i+1. Column 127 is zero padding.
    m = np.zeros((P, P), dtype=np.float32)
    i = np.arange(HO)
    m[i, i] = 1.0
    m[i + 1, i] = 1.0
    return m


def _build(repeat: int = 1):
    import concourse.bacc as bacc
    import concourse.mybir as mybir
    import concourse.tile as tile

    f32 = mybir.dt.float32
    f32r = mybir.dt.float32r

    nc = bacc.Bacc("TRN2", target_bir_lowering=False, debug=False, num_devices=B)
    x_t = nc.dram_tensor("x", [C, H, W], f32r, kind="ExternalInput")
    mv_t = nc.dram_tensor("mv", [P, P], f32r, kind="ExternalInput")
    # out[i, c, j]: i-major so each partition's store span is contiguous DRAM
    out_t = nc.dram_tensor("out", [P, C, P], f32, kind="ExternalOutput")

    # [cg, p, g, (e w)]: plane c = G*cg + g, partition p holds rows 4p..4p+3
    x_ap = x_t.ap().rearrange("(cg g) (p e) w -> cg p g (e w)", g=G, p=P)
    # [sg, i, (c j)]: store group sg covers planes sg*SG .. sg*SG+SG-1
    out_ap = out_t.ap().rearrange("i (sg c) j -> sg i (c j)", sg=C // SG)

    with tile.TileContext(nc) as tc:
        with (
            tc.tile_pool(name="consts", bufs=1) as consts,
            tc.tile_pool(name="xin", bufs=4) as xin,
            tc.tile_pool(name="vpsum", bufs=8, space="PSUM") as vpsum,
            tc.tile_pool(name="tmp", bufs=6) as tmp,
            tc.tile_pool(name="stage", bufs=2) as stage,
        ):
            mv = consts.tile([P, P], f32r)
            nc.sync.dma_start(mv, mv_t.ap())
            for _ in range(repeat):
                st = None
                for c in range(C):
                    g = c % G
                    if g == 0:
                        xt = xin.tile([P, G * E * W], f32r)
                        nc.sync.dma_start(
                            xt[:].rearrange("p (g f) -> p g f", g=G), x_ap[c // G]
                        )
                    if c % SG == 0:
                        st = stage.tile([P, SG * P], f32)
                    v = vpsum.tile([P, W], f32)
                    for e in range(E):
                        nc.tensor.matmul(
                            v,
                            mv,
                            xt[:, (g * E + e) * W : (g * E + e + 1) * W],
                            start=(e == 0),
                            stop=(e == E - 1),
                        )
                    v2 = v[:].rearrange("i (u two) -> i u two", two=2)
                    a0 = tmp.tile([P, W // 2], f32)
                    nc.vector.tensor_copy(a0, v2[:, :, 0])
                    a = tmp.tile([P, W // 2], f32)
                    nc.vector.tensor_add(a, v2[:, :, 1], a0)
                    a2 = a[:].rearrange("i (m two) -> i m two", two=2)
                    b = tmp.tile([P, W // 4], f32)
                    nc.vector.tensor_add(b, a2[:, :, 0], a2[:, :, 1])
                    o = (c % SG) * P
                    nc.vector.tensor_add(
                        st[:, o : o + WO], b[:, 0:WO], b[:, 1 : WO + 1]
                    )
                    if (c + 1) % SG == 0:
                        # pad column WO of each plane is stored but never read
                        # by the host; it stays unwritten SBUF
                        nc.scalar.dma_start(out_ap[c // SG], st)
    nc.compile()
    return nc


def kernel(x: np.ndarray) -> np.ndarray:
    from concourse import bass_utils

    nc = _CACHE.get("nc")
    if nc is None:
        nc = _CACHE["nc"] = _build()
    x = np.ascontiguousarray(np.asarray(x, dtype=np.float32))
    assert x.shape == (B, C, H, W)
    mv = _pool_matrix()
    in_maps = [{"x": x[b], "mv": mv} for b in range(B)]
    res = bass_utils.run_bass_kernel_spmd(nc, in_maps, core_ids=list(range(B)))
    # out[i, c, j] -> [c, i, j], drop the i/j pads
    return np.stack(
        [res.results[b]["out"].transpose(1, 0, 2)[:, :HO, :WO] for b in range(B)],
        axis=0,
    )


# revision 5
# speedup vs baseline: 1.0424x; 1.0424x over previous
"""Trainium2 Bass kernel: per-channel 8x8 box-sum pooling, stride 4 (NCHW).

Input  x: (8, 32, 512, 512) f32  ->  output (8, 32, 127, 127) f32.

Sharding: data-parallel over the batch dim — image b runs on NeuronCore b
(zero communication).

Per core (32 channel planes of 512 x 512):

  1. Input DMA, one plane per dma_start: SBUF layout [128, e*512] with
     partition p holding input rows 4p..4p+3 — each partition's span is
     8 KiB of *contiguous* DRAM, so descriptors are 8 KiB and each 1-MiB
     plane stripes across all 16 DMA engines (64-KiB chunk round-robin).
  2. Vertical pooling on the tensor engine in f32r: with 4 consecutive rows
     per partition, window i covers exactly partitions i and i+1, for every
     row-chunk e. So all 4 accumulating matmuls share one bidiagonal weight
     matrix M[p, i] = (p==i) | (p==i+1):  V[i, w] = sum_e (M.T @ X_e)[i, w]
     = sum_{dh<8} x[4i+dh, w].  f32r rounds the data operand's mantissa
     (weights are exact 0/1); rel err ~1e-4 vs the 2e-2 gate.
  3. Horizontal pooling on the vector engine, reading PSUM [128, 512]:
     copy evens to SBUF, a[u]=V[2u]+V[2u+1] (one PSUM operand max per DVE
     op); b[m]=a[2m]+a[2m+1]; out[i,j]=b[j]+b[j+1].
  4. The final add writes into an SBUF staging tile (row i of plane c at
     free offset k*128 for the k-th plane of the group). Group stores of
     16/14/2 planes write DRAM out[i, c, j] — i-major, so each partition's
     group span is contiguous -> 8-KiB-scale descriptors striped across all
     DMA engines (instead of 32 per-plane 65-KiB stores that all land on
     DMA engine 0), and the 1-MiB group-0 store overlaps the input stream
     while the last store is only 128 KiB of tail. The host transposes
     [128, 32, 128] -> [32, 127, 127].

Row 127 of V (weight column 127 is all-zero) and output column 127 carry
zeros/garbage; the host slices both pads off.
"""

import numpy as np

B, C, H, W = 8, 32, 512, 512
KS, ST = 8, 4
HO = (H - KS) // ST + 1  # 127
WO = (W - KS) // ST + 1  # 127
P = 128
E = H // P  # 4 rows per partition
GROUPS = ((0, 16), (16, 30), (30, 32))  # output store groups [a, b)

_CACHE: dict = {}


def _pool_matrix() -> np.ndarray:
    # M[p, i] = 1 iff output row i's 8-row window covers partition p's rows
    # (4p..4p+3), i.e. p == i or p == i+1. Column 127 is zero padding.
    m = np.zeros((P, P), dtype=np.float32)
    i = np.arange(HO)
    m[i, i] = 1.0
    m[i + 1, i] = 1.0
    return m


def _build(repeat: int = 1):
    import concourse.bacc as bacc
    import concourse.mybir as mybir
    import concourse.tile as tile

    f32 = mybir.dt.float32
    f32r = mybir.dt.float32r

    nc = bacc.Bacc("TRN2", target_bir_lowering=False, debug=False, num_devices=B)
    x_t = nc.dram_tensor("x", [C, H, W], f32r, kind="ExternalInput")
    mv_t = nc.dram_tensor("mv", [P, P], f32r, kind="ExternalInput")
    # out[i, c, j]: i-major so each partition's store span is contiguous DRAM
    out_t = nc.dram_tensor("out", [P, C, P], f32, kind="ExternalOutput")

    # [c, p, (e w)]: partition p holds rows 4p..4p+3 of plane c
    x_ap = x_t.ap().rearrange("c (p e) w -> c p (e w)", p=P)
    out_ap = out_t.ap()

    with tile.TileContext(nc) as tc:
        with (
            tc.tile_pool(name="consts", bufs=1) as consts,
            tc.tile_pool(name="xin", bufs=8) as xin,
            tc.tile_pool(name="vpsum", bufs=8, space="PSUM") as vpsum,
            tc.tile_pool(name="tmp", bufs=6) as tmp,
            tc.tile_pool(name="stage", bufs=len(GROUPS)) as stage,
        ):
            mv = consts.tile([P, P], f32r)
            nc.sync.dma_start(mv, mv_t.ap())
            for _ in range(repeat):
                st = None
                gi = 0
                for c in range(C):
                    ga, gb = GROUPS[gi]
                    xt = xin.tile([P, E * W], f32r)
                    nc.sync.dma_start(xt, x_ap[c])
                    if c == ga:
                        st = stage.tile([P, (gb - ga) * P], f32)
                    v = vpsum.tile([P, W], f32)
                    for e in range(E):
                        nc.tensor.matmul(
                            v,
                            mv,
                            xt[:, e * W : (e + 1) * W],
                            start=(e == 0),
                            stop=(e == E - 1),
                        )
                    v2 = v[:].rearrange("i (u two) -> i u two", two=2)
                    a0 = tmp.tile([P, W // 2], f32)
                    nc.vector.tensor_copy(a0, v2[:, :, 0])
                    a = tmp.tile([P, W // 2], f32)
                    nc.vector.tensor_add(a, v2[:, :, 1], a0)
                    a2 = a[:].rearrange("i (m two) -> i m two", two=2)
                    b = tmp.tile([P, W // 4], f32)
                    nc.vector.tensor_add(b, a2[:, :, 0], a2[:, :, 1])
                    o = (c - ga) * P
                    nc.vector.tensor_add(
                        st[:, o : o + WO], b[:, 0:WO], b[:, 1 : WO + 1]
                    )
                    if c + 1 == gb:
                        # pad column WO of each plane is stored but never
                        # read by the host; it stays unwritten SBUF
                        nc.scalar.dma_start(
                            out_ap[:, ga:gb, :],
                            st[:].rearrange("i (c j) -> i c j", j=P),
                        )
                        gi = (gi + 1) % len(GROUPS)
    nc.compile()
    return nc


def kernel(x: np.ndarray) -> np.ndarray:
    from concourse import bass_utils

    nc = _CACHE.get("nc")
    if nc is None:
        nc = _CACHE["nc"] = _build()
    x = np.ascontiguousarray(np.asarray(x, dtype=np.float32))
    assert x.shape == (B, C, H, W)
    mv = _pool_matrix()
    in_maps = [{"x": x[b], "mv": mv} for b in range(B)]
    res = bass_utils.run_bass_kernel_spmd(nc, in_maps, core_ids=list(range(B)))
    # out[i, c, j] -> [c, i, j], drop the i/j pads
    return np.stack(
        [res.results[b]["out"].transpose(1, 0, 2)[:, :HO, :WO] for b in range(B)],
        axis=0,
    )
